# revision 24
# baseline (speedup 1.0000x reference)
"""Trainium2 Bass kernel for nn_ReadoutBlock (gated graph readout).

Computation (see module docstring math):
    gi    = global_information[batch]                  # [N, D] gather
    gw    = sigmoid([x, gi] @ mlp_w.T + mlp_b)         # [N, D]
    gf_new = segment_sum(gw * x, batch, G)             # [G, D]
    gf    = GRUCell(gf_new, global_information)        # [G, D]
    loss  = mean_g(segment_sum(|gw|_2, batch) / max(counts, 1))

Distribution strategy: shard nodes BY GRAPH ID (core k owns all nodes whose
graph g satisfies g % 8 == k, i.e. all graphs {g : g & 7 == k}).  Each core
then scatters into only 128 graphs, which fits a single PSUM accumulator
tile, needs no collectives, and the GRU runs shard-locally on the core's own
128 graphs.  The host only partitions/pads node arrays and re-interleaves the
[128, D] per-core outputs.

Device pipeline per 128-node tile (nodes on PSUM/SBUF partitions):
    z   = x @ W1 + U[b]      two accumulating matmuls; U = G_info @ W2 + b_mlp
                             gather realised as onehotT.T @ U_k on the PE
    gw  = sigmoid(z)         ScalarE, PSUM -> SBUF bf16
    v   = gw * x             GpSimd elementwise
    n2  = sum_d gw^2         DVE tensor_tensor_reduce (fused square+reduce)
    nrm = sqrt(n2)           ScalarE, batched per 2048-node group
    A  += onehot.T @ [v | nrm | 1]   scatter matmul, PSUM-resident accumulator
"""

import functools
import os
from contextlib import ExitStack

import numpy as np
import ml_dtypes

BF16 = np.float16

N_CORES = 8
G = 1024
D = 128
G_LOC = G // N_CORES          # graphs per core
GROUP = 2048                  # nodes per group (16 tiles of 128)
TILES_PER_GROUP = GROUP // 128

# test.py sets this via profile(); harness path never touches it.
_PROFILE = {"trace": False, "last": None}


def _round_up(x, m):
    return (x + m - 1) // m * m


@functools.lru_cache(maxsize=2)
def _build_program(nmax: int):
    import concourse.bass as bass
    import concourse.tile as tile
    from concourse import bacc, mybir
    from concourse.tile import add_dep_helper

    f32 = mybir.dt.float32
    bf16 = mybir.dt.float16  # fp16: 8x less rounding than bf16, same speed
    f8 = mybir.dt.float8e4   # one-hots: 0/1 exact, half the HBM bytes
    i16 = mybir.dt.int16
    AF = mybir.ActivationFunctionType
    ALU = mybir.AluOpType

    n_groups = nmax // GROUP
    T = nmax // 128  # total 128-node tiles

    nc = bacc.Bacc(
        "TRN2",
        target_bir_lowering=False,
        debug=False,
        enable_asserts=True,
        num_devices=N_CORES,
    )

    # ---- DRAM I/O ----
    def din(name, shape, dt):
        return nc.dram_tensor(name, shape, dt, kind="ExternalInput").ap()

    xT_full = din("xT", [128, nmax], bf16)       # x transposed [feat, node]
    x_wrap = din("x_wrap", [128, nmax // 128, D], bf16)  # [n%128, tile, feat]
    oh_wrap = din("oh_wrap", [128, nmax // 128, G_LOC], f8)  # wrapped one-hot
    ohT = din("ohT", [128, nmax], f8)            # one-hot(batch) transposed
    u_k = din("u_k", [G_LOC, D], bf16)           # U rows of this core's graphs
    w1 = din("w1", [D, D], bf16)                 # mlp_w.T[:D]  ([feat_in, d_out])
    gkT16 = din("gkT16", [D, G_LOC], bf16)       # h^T for GRU matmuls
    gkTf = din("gkTf", [D, G_LOC], f32)          # h^T fp32 for the blend
    wihT = din("wihT", [D, 3 * D], bf16)         # w_ih.T
    whhT = din("whhT", [D, 3 * D], bf16)         # w_hh.T
    biases = din("biases", [128, 4], f32)        # [b_r, b_z, b_hhn, b_ihn]
    ident = din("ident", [128, 128], bf16)
    ones_col = din("ones_col", [128, 1], f32)

    gfT_out = nc.dram_tensor("gfT", [D, G_LOC], f32, kind="ExternalOutput").ap()
    loss_out = nc.dram_tensor("loss", [1, 1], f32, kind="ExternalOutput").ap()

    with tile.TileContext(nc) as tc, ExitStack() as ctx:
        consts = ctx.enter_context(tc.tile_pool(name="consts", bufs=1))
        xT_pool = ctx.enter_context(tc.tile_pool(name="xT", bufs=8))
        xn_pool = ctx.enter_context(tc.tile_pool(name="xn", bufs=8))
        bT_pool = ctx.enter_context(tc.tile_pool(name="bT", bufs=2))
        ohT_pool = ctx.enter_context(tc.tile_pool(name="ohT", bufs=8))
        oh_pool = ctx.enter_context(tc.tile_pool(name="oh", bufs=10))
        gw_pool = ctx.enter_context(tc.tile_pool(name="gw", bufs=6))
        scr_pool = ctx.enter_context(tc.tile_pool(name="scr", bufs=6))
        v_pool = ctx.enter_context(tc.tile_pool(name="v", bufs=3))
        n2_pool = ctx.enter_context(tc.tile_pool(name="n2", bufs=3))
        sb_pool = ctx.enter_context(tc.tile_pool(name="sb", bufs=2))
        z_pool = ctx.enter_context(tc.tile_pool(name="z", bufs=4, space="PSUM"))
        a_pool = ctx.enter_context(tc.tile_pool(name="acc", bufs=1, space="PSUM"))
        g_pool = ctx.enter_context(tc.tile_pool(name="gpsum", bufs=2, space="PSUM"))

        # ---- constants into SBUF ----
        def cload(ap_dram, shape, dt):
            t = consts.tile(shape, dt, tag=ap_dram.tensor.name)
            nc.sync.dma_start(t[:], ap_dram)
            return t

        w1_sb = cload(w1, [D, D], bf16)
        uk_sb = cload(u_k, [G_LOC, D], bf16)
        ident_sb = cload(ident, [128, 128], bf16)
        gkT16_sb = cload(gkT16, [D, G_LOC], bf16)
        gkTf_sb = cload(gkTf, [D, G_LOC], f32)
        wihT_sb = cload(wihT, [D, 3 * D], bf16)
        whhT_sb = cload(whhT, [D, 3 * D], bf16)
        bias_sb = cload(biases, [128, 4], f32)
        ones_sb = cload(ones_col, [128, 1], f32)

        # Scatter accumulators (two, to break the WAW chain in half):
        # [g_local, 0:128]=gf_new, 128=seg_norm, 129=counts
        A0 = a_pool.tile([G_LOC, 130], f32, tag="acc0")
        A1 = a_pool.tile([G_LOC, 130], f32, tag="acc1")

        first_mm3 = [True, True]
        SG = 4  # groups per supergroup (batches the Sqrt to limit ACT table loads)
        sg_bounds = [(i, min(i + SG, n_groups)) for i in range(0, n_groups, SG)]
        for g0, g1 in sg_bounds:
            gcnt = g1 - g0
            tiles_sg = gcnt * TILES_PER_GROUP
            vg = v_pool.tile([128, SG * TILES_PER_GROUP, 130], bf16)
            n2g = n2_pool.tile([128, SG * TILES_PER_GROUP], f32)
            ohgs = {}

            for g in range(g0, g1):
                lo = g * GROUP
                toff = (g - g0) * TILES_PER_GROUP
                t0 = g * TILES_PER_GROUP
                xTa = xT_pool.tile([128, GROUP // 2], bf16, tag="xTa")
                nc.sync.dma_start(xTa[:], xT_full[:, lo : lo + GROUP // 2])
                xTb = xT_pool.tile([128, GROUP // 2], bf16, tag="xTb")
                nc.sync.dma_start(
                    xTb[:], xT_full[:, lo + GROUP // 2 : lo + GROUP])
                xT_half = (xTa, xTb)
                xng = xn_pool.tile([128, TILES_PER_GROUP, 128], bf16)
                nc.sync.dma_start(xng[:], x_wrap[:, t0 : t0 + TILES_PER_GROUP, :])
                ohTg = ohT_pool.tile([128, GROUP], f8)
                nc.sync.dma_start(ohTg[:], ohT[:, lo : lo + GROUP])
                ohg = oh_pool.tile([128, TILES_PER_GROUP, G_LOC], f8)
                nc.sync.dma_start(ohg[:], oh_wrap[:, t0 : t0 + TILES_PER_GROUP, :])
                ohgs[g] = ohg

                for st in range(4):
                    z = z_pool.tile([128, 4, 128], f32)
                    prev_mm2 = None
                    for s in range(4):
                        t_loc = st * 4 + s  # tile within group
                        xTh = xT_half[t_loc // 8]
                        thl = t_loc % 8
                        mm1 = nc.tensor.matmul(
                            z[:, s, :],
                            xTh[:, thl * 128 : (thl + 1) * 128],
                            w1_sb[:],
                            start=True,
                            stop=False,
                        )
                        if prev_mm2 is not None:
                            # PSUM has_written bits are cleared bank-wide by a
                            # start=True matmul; keep each subtile's accumulate
                            # pair ordered before the next pair's start.
                            add_dep_helper(mm1.ins, prev_mm2.ins, sync=False,
                                           reason="psum start/accum ordering")
                        mm2 = nc.tensor.matmul(
                            z[:, s, :],
                            ohTg[:, t_loc * 128 : (t_loc + 1) * 128],
                            uk_sb[:],
                            start=False,
                            stop=True,
                        )
                        prev_mm2 = mm2

                    gw = gw_pool.tile([128, 4, 128], bf16)
                    nc.scalar.activation(gw[:], z[:], AF.Sigmoid)
                    nc.vector.tensor_tensor(
                        vg[:, toff + st * 4 : toff + st * 4 + 4, 0:128],
                        gw[:],
                        xng[:, st * 4 : st * 4 + 4, :],
                        op=ALU.mult,
                    )
                    t_sg = toff + st * 4
                    scr = scr_pool.tile([128, 4, 128], bf16)
                    sq_eng = nc.gpsimd if st % 2 == 1 else nc.vector
                    sq_eng.tensor_tensor(scr[:], gw[:], gw[:], op=ALU.mult)
                    sfold = scr_pool.tile([128, 4, 64], bf16, tag="sfold")
                    nc.vector.tensor_tensor(
                        sfold[:], scr[:, :, 0:64], scr[:, :, 64:128], op=ALU.add
                    )
                    nc.vector.tensor_reduce(
                        n2g[:, t_sg : t_sg + 4],
                        sfold[:],
                        axis=mybir.AxisListType.X,
                        op=ALU.add,
                    )

            # norm + count columns in two halves so scatter can start earlier.
            half = tiles_sg // 2 if tiles_sg > 1 else tiles_sg
            for h0, h1 in ((0, half), (half, tiles_sg)):
                if h0 == h1:
                    continue
                nrm_ap = vg[:, h0:h1, 128:129].rearrange("p t o -> p (t o)")
                one_ap = vg[:, h0:h1, 129:130].rearrange("p t o -> p (t o)")
                nc.scalar.activation(nrm_ap, n2g[:, h0:h1], AF.Sqrt)
                nc.gpsimd.memset(one_ap, 1.0)

            last_sg = g1 == n_groups
            for g in range(g0, g1):
                toff = (g - g0) * TILES_PER_GROUP
                for t_loc in range(TILES_PER_GROUP):
                    par = (toff + t_loc) % 2
                    acc = A0 if par == 0 else A1
                    nc.tensor.matmul(
                        acc[:, 0:130],
                        ohgs[g][:, t_loc, :],
                        vg[:, toff + t_loc, :],
                        start=first_mm3[par],
                        stop=last_sg
                        and g == g1 - 1
                        and t_loc >= TILES_PER_GROUP - 2,
                    )
                    first_mm3[par] = False

        # ---- merge the two accumulators ----
        A1c = sb_pool.tile([G_LOC, 130], f32, tag="a1c")
        nc.vector.tensor_copy(A1c[:], A1[:])
        As = sb_pool.tile([G_LOC, 130], f32, tag="asum")
        nc.vector.tensor_tensor(As[:], A0[:], A1c[:], op=ALU.add)

        # ---- GRU (PyTorch GRUCell math) on this core's 128 graphs ----
        gfn16 = sb_pool.tile([G_LOC, D], bf16, tag="gfn16")
        nc.vector.tensor_copy(gfn16[:], As[:, 0:128])
        tr_ps = g_pool.tile([D, G_LOC], bf16, tag="gru")
        nc.tensor.transpose(tr_ps[:], gfn16[:], ident_sb[:])
        gfnT = sb_pool.tile([D, G_LOC], bf16, tag="gfnT")
        nc.vector.tensor_copy(gfnT[:], tr_ps[:])

        gates = []
        for c in (0, 1):  # r, z
            ps = g_pool.tile([128, G_LOC], f32, tag="gru")
            nc.tensor.matmul(ps[:], wihT_sb[:, c * 128 : (c + 1) * 128], gfnT[:],
                             start=True, stop=False)
            nc.tensor.matmul(ps[:], whhT_sb[:, c * 128 : (c + 1) * 128], gkT16_sb[:],
                             start=False, stop=True)
            gate = sb_pool.tile([128, G_LOC], f32, tag=f"gate{c}")
            nc.scalar.activation(gate[:], ps[:], AF.Sigmoid, bias=bias_sb[:, c : c + 1])
            gates.append(gate)
        r_sb, z_sb = gates

        ps_in = g_pool.tile([128, G_LOC], f32, tag="gru")
        nc.tensor.matmul(ps_in[:], wihT_sb[:, 256:384], gfnT[:], start=True, stop=True)
        ps_hn = g_pool.tile([128, G_LOC], f32, tag="gru")
        nc.tensor.matmul(ps_hn[:], whhT_sb[:, 256:384], gkT16_sb[:], start=True, stop=True)

        hnb = sb_pool.tile([128, G_LOC], f32, tag="hnb")
        nc.vector.tensor_scalar(hnb[:], ps_hn[:], bias_sb[:, 2:3], None, op0=ALU.add)
        rh = sb_pool.tile([128, G_LOC], f32, tag="rh")
        nc.vector.tensor_tensor(rh[:], r_sb[:], hnb[:], op=ALU.mult)
        tsum = sb_pool.tile([128, G_LOC], f32, tag="tsum")
        nc.vector.tensor_tensor(tsum[:], rh[:], ps_in[:], op=ALU.add)
        n_sb = sb_pool.tile([128, G_LOC], f32, tag="nsb")
        nc.scalar.activation(n_sb[:], tsum[:], AF.Tanh, bias=bias_sb[:, 3:4])

        d1 = sb_pool.tile([128, G_LOC], f32, tag="d1")
        nc.vector.tensor_tensor(d1[:], gkTf_sb[:], n_sb[:], op=ALU.subtract)
        d2 = sb_pool.tile([128, G_LOC], f32, tag="d2")
        nc.vector.tensor_tensor(d2[:], z_sb[:], d1[:], op=ALU.mult)
        gfT_sb = sb_pool.tile([128, G_LOC], f32, tag="gfT")
        nc.vector.tensor_tensor(gfT_sb[:], n_sb[:], d2[:], op=ALU.add)
        nc.sync.dma_start(gfT_out, gfT_sb[:])

        # ---- gate-loss partial: sum_g seg_norm / max(counts, 1) ----
        cnt_m = sb_pool.tile([128, 1], f32, tag="cntm")
        nc.vector.tensor_scalar_max(cnt_m[:], As[:, 129:130], 1.0)
        rec = sb_pool.tile([128, 1], f32, tag="rec")
        nc.vector.reciprocal(rec[:], cnt_m[:])
        ratio = sb_pool.tile([128, 1], f32, tag="ratio")
        nc.vector.tensor_tensor(ratio[:], As[:, 128:129], rec[:], op=ALU.mult)
        loss_ps = g_pool.tile([1, 1], f32, tag="gru")
        nc.tensor.matmul(loss_ps[:], ratio[:], ones_sb[:], start=True, stop=True)
        loss_sb = sb_pool.tile([1, 1], f32, tag="losssb")
        nc.vector.tensor_copy(loss_sb[:], loss_ps[:])
        nc.sync.dma_start(loss_out, loss_sb[:])

    nc.compile()
    return nc


def _prepare_inputs(x, batch, global_information, mlp_w, mlp_b, w_ih, w_hh, b_ih, b_hh):
    """Shard by graph id, pad, cast; returns (nmax, list-of-8 input dicts)."""
    x = np.asarray(x)
    batch = np.asarray(batch)
    gi = np.asarray(global_information, dtype=np.float32)
    mlp_w = np.asarray(mlp_w, dtype=np.float32)
    mlp_b = np.asarray(mlp_b, dtype=np.float32)
    w_ih = np.asarray(w_ih, dtype=np.float32)
    w_hh = np.asarray(w_hh, dtype=np.float32)
    b_ih = np.asarray(b_ih, dtype=np.float32)
    b_hh = np.asarray(b_hh, dtype=np.float32)

    b64 = batch.astype(np.int64)
    # U = G_info @ W2 + b   (W2 = mlp_w.T[D:, :]); W1 = mlp_w.T[:D, :]
    w1_np = np.ascontiguousarray(mlp_w[:, :D].T)
    w2_np = mlp_w[:, D:].T
    u_full = gi.astype(np.float32) @ w2_np + mlp_b[None, :]

    core_of = b64 & 7
    g_loc_all = (b64 >> 3).astype(np.int16)
    counts = np.bincount(core_of, minlength=N_CORES)
    nmax = max(GROUP, _round_up(int(counts.max()), GROUP))

    bias4 = np.stack(
        [
            b_ih[0:128] + b_hh[0:128],
            b_ih[128:256] + b_hh[128:256],
            b_hh[256:384],
            b_ih[256:384],
        ],
        axis=1,
    ).astype(np.float32)

    ident = np.eye(128, dtype=BF16)
    ones_col = np.ones((128, 1), dtype=np.float32)
    wihT_np = np.ascontiguousarray(w_ih.T).astype(BF16)
    whhT_np = np.ascontiguousarray(w_hh.T).astype(BF16)

    in_maps = []
    for k in range(N_CORES):
        idx = np.nonzero(core_of == k)[0]
        cnt = idx.size
        x_k = np.zeros((nmax, D), dtype=BF16)
        x_k[:cnt] = x[idx]
        xT_np = np.ascontiguousarray(x_k.T)
        x_wrap = np.ascontiguousarray(
            x_k.reshape(nmax // 128, 128, D).transpose(1, 0, 2))
        gl = g_loc_all[idx].astype(np.int64)
        oh_nat = np.zeros((nmax, G_LOC), dtype=ml_dtypes.float8_e4m3)
        oh_nat[np.arange(cnt), gl] = 1.0
        oh_wrap = np.ascontiguousarray(
            oh_nat.reshape(nmax // 128, 128, G_LOC).transpose(1, 0, 2))
        ohT_np = np.zeros((128, nmax), dtype=ml_dtypes.float8_e4m3)
        ohT_np[gl, np.arange(cnt)] = 1.0
        gk = gi[k::8]  # [128, D] this core's graph states
        in_maps.append(
            {
                "xT": xT_np,
                "x_wrap": x_wrap,
                "oh_wrap": oh_wrap,
                "ohT": ohT_np,
                "u_k": u_full[k::8].astype(BF16),
                "w1": w1_np.astype(BF16),
                "gkT16": np.ascontiguousarray(gk.T).astype(BF16),
                "gkTf": np.ascontiguousarray(gk.T).astype(np.float32),
                "wihT": wihT_np,
                "whhT": whhT_np,
                "biases": bias4,
                "ident": ident,
                "ones_col": ones_col,
            }
        )
    return nmax, in_maps


def kernel(x, batch, global_information, mlp_w, mlp_b, w_ih, w_hh, b_ih, b_hh):
    from concourse import bass_utils

    nmax, in_maps = _prepare_inputs(
        x, batch, global_information, mlp_w, mlp_b, w_ih, w_hh, b_ih, b_hh
    )
    nc = _build_program(nmax)
    res = bass_utils.run_bass_kernel_spmd(
        nc,
        in_maps,
        core_ids=list(range(N_CORES)),
        trace=_PROFILE["trace"],
    )
    _PROFILE["last"] = res

    gf = np.empty((G, D), dtype=np.float32)
    loss_sum = 0.0
    for k in range(N_CORES):
        gf[k::8] = res.results[k]["gfT"].T
        loss_sum += float(res.results[k]["loss"][0, 0])
    gate_loss = np.float32(loss_sum / G)
    return gf, gate_loss


# revision 25
# speedup vs baseline: 1.1860x; 1.1860x over previous
"""Trainium2 Bass kernel for nn_ReadoutBlock (gated graph readout).

Computation (see module docstring math):
    gi    = global_information[batch]                  # [N, D] gather
    gw    = sigmoid([x, gi] @ mlp_w.T + mlp_b)         # [N, D]
    gf_new = segment_sum(gw * x, batch, G)             # [G, D]
    gf    = GRUCell(gf_new, global_information)        # [G, D]
    loss  = mean_g(segment_sum(|gw|_2, batch) / max(counts, 1))

Distribution strategy: shard nodes BY GRAPH ID (core k owns all nodes whose
graph g satisfies g % 8 == k, i.e. all graphs {g : g & 7 == k}).  Each core
then scatters into only 128 graphs, which fits a single PSUM accumulator
tile, needs no collectives, and the GRU runs shard-locally on the core's own
128 graphs.  The host only partitions/pads node arrays and re-interleaves the
[128, D] per-core outputs.

Device pipeline per 128-node tile (nodes on PSUM/SBUF partitions):
    z   = x @ W1 + U[b]      two accumulating matmuls; U = G_info @ W2 + b_mlp
                             gather realised as onehotT.T @ U_k on the PE
    gw  = sigmoid(z)         ScalarE, PSUM -> SBUF bf16
    v   = gw * x             GpSimd elementwise
    n2  = sum_d gw^2         DVE tensor_tensor_reduce (fused square+reduce)
    nrm = sqrt(n2)           ScalarE, batched per 2048-node group
    A  += onehot.T @ [v | nrm | 1]   scatter matmul, PSUM-resident accumulator
"""

import functools
import os
from contextlib import ExitStack

import numpy as np
import ml_dtypes

BF16 = np.float16

N_CORES = 8
G = 1024
D = 128
G_LOC = G // N_CORES          # graphs per core
GROUP = 2048                  # nodes per group (16 tiles of 128)
TILES_PER_GROUP = GROUP // 128

# test.py sets this via profile(); harness path never touches it.
_PROFILE = {"trace": False, "last": None}


def _round_up(x, m):
    return (x + m - 1) // m * m


@functools.lru_cache(maxsize=2)
def _build_program(nmax: int):
    import concourse.bass as bass
    import concourse.tile as tile
    from concourse import bacc, mybir
    from concourse.tile import add_dep_helper

    f32 = mybir.dt.float32
    bf16 = mybir.dt.float16  # fp16: 8x less rounding than bf16, same speed
    f8 = mybir.dt.float8e4   # one-hots: 0/1 exact, half the HBM bytes
    i16 = mybir.dt.int16
    AF = mybir.ActivationFunctionType
    ALU = mybir.AluOpType

    n_groups = nmax // GROUP
    T = nmax // 128  # total 128-node tiles

    nc = bacc.Bacc(
        "TRN2",
        target_bir_lowering=False,
        debug=False,
        enable_asserts=True,
        num_devices=N_CORES,
    )

    # ---- DRAM I/O ----
    def din(name, shape, dt):
        return nc.dram_tensor(name, shape, dt, kind="ExternalInput").ap()

    xT_full = din("xT", [128, nmax], bf16)       # x transposed [feat, node]
    x_wrap = din("x_wrap", [128, nmax // 128, D], bf16)  # [n%128, tile, feat]
    oh_wrap = din("oh_wrap", [128, nmax // 128, G_LOC], f8)  # wrapped one-hot
    ohT = din("ohT", [128, nmax], f8)            # one-hot(batch) transposed
    u_k = din("u_k", [G_LOC, D], bf16)           # U rows of this core's graphs
    w1 = din("w1", [D, D], bf16)                 # mlp_w.T[:D]  ([feat_in, d_out])
    gkT16 = din("gkT16", [D, G_LOC], bf16)       # h^T for GRU matmuls
    gkTf = din("gkTf", [D, G_LOC], f32)          # h^T fp32 for the blend
    wihT = din("wihT", [D, 3 * D], bf16)         # w_ih.T
    whhT = din("whhT", [D, 3 * D], bf16)         # w_hh.T
    biases = din("biases", [128, 4], f32)        # [b_r, b_z, b_hhn, b_ihn]
    ident = din("ident", [128, 128], bf16)
    ones_col = din("ones_col", [128, 1], f32)

    gfT_out = nc.dram_tensor("gfT", [D, G_LOC], f32, kind="ExternalOutput").ap()
    loss_out = nc.dram_tensor("loss", [1, 1], f32, kind="ExternalOutput").ap()

    with tile.TileContext(nc) as tc, ExitStack() as ctx:
        consts = ctx.enter_context(tc.tile_pool(name="consts", bufs=1))
        xT_pool = ctx.enter_context(tc.tile_pool(name="xT", bufs=8))
        xn_pool = ctx.enter_context(tc.tile_pool(name="xn", bufs=8))
        bT_pool = ctx.enter_context(tc.tile_pool(name="bT", bufs=2))
        ohT_pool = ctx.enter_context(tc.tile_pool(name="ohT", bufs=8))
        oh_pool = ctx.enter_context(tc.tile_pool(name="oh", bufs=10))
        gw_pool = ctx.enter_context(tc.tile_pool(name="gw", bufs=6))
        scr_pool = ctx.enter_context(tc.tile_pool(name="scr", bufs=6))
        v_pool = ctx.enter_context(tc.tile_pool(name="v", bufs=3))
        n2_pool = ctx.enter_context(tc.tile_pool(name="n2", bufs=3))
        sb_pool = ctx.enter_context(tc.tile_pool(name="sb", bufs=2))
        z_pool = ctx.enter_context(tc.tile_pool(name="z", bufs=4, space="PSUM"))
        a_pool = ctx.enter_context(tc.tile_pool(name="acc", bufs=1, space="PSUM"))
        g_pool = ctx.enter_context(tc.tile_pool(name="gpsum", bufs=2, space="PSUM"))

        # ---- constants into SBUF ----
        def cload(ap_dram, shape, dt):
            t = consts.tile(shape, dt, tag=ap_dram.tensor.name)
            nc.sync.dma_start(t[:], ap_dram)
            return t

        w1_sb = cload(w1, [D, D], bf16)
        uk_sb = cload(u_k, [G_LOC, D], bf16)
        ident_sb = cload(ident, [128, 128], bf16)
        gkT16_sb = cload(gkT16, [D, G_LOC], bf16)
        gkTf_sb = cload(gkTf, [D, G_LOC], f32)
        wihT_sb = cload(wihT, [D, 3 * D], bf16)
        whhT_sb = cload(whhT, [D, 3 * D], bf16)
        bias_sb = cload(biases, [128, 4], f32)
        ones_sb = cload(ones_col, [128, 1], f32)

        # Scatter accumulators (two, to break the WAW chain in half):
        # [g_local, 0:128]=gf_new, 128=seg_norm, 129=counts
        A0 = a_pool.tile([G_LOC, 130], f32, tag="acc0")
        A1 = a_pool.tile([G_LOC, 130], f32, tag="acc1")

        first_mm3 = [True, True]
        SG = 4  # groups per supergroup (batches the Sqrt to limit ACT table loads)
        sg_bounds = [(i, min(i + SG, n_groups)) for i in range(0, n_groups, SG)]
        for g0, g1 in sg_bounds:
            gcnt = g1 - g0
            tiles_sg = gcnt * TILES_PER_GROUP
            vg = v_pool.tile([128, SG * TILES_PER_GROUP, 130], bf16)
            n2g = n2_pool.tile([128, SG * TILES_PER_GROUP], f32)
            ohgs = {}

            for g in range(g0, g1):
                lo = g * GROUP
                toff = (g - g0) * TILES_PER_GROUP
                t0 = g * TILES_PER_GROUP
                xTa = xT_pool.tile([128, GROUP // 2], bf16, tag="xTa")
                nc.sync.dma_start(xTa[:], xT_full[:, lo : lo + GROUP // 2])
                xTb = xT_pool.tile([128, GROUP // 2], bf16, tag="xTb")
                nc.sync.dma_start(
                    xTb[:], xT_full[:, lo + GROUP // 2 : lo + GROUP])
                xT_half = (xTa, xTb)
                xng = xn_pool.tile([128, TILES_PER_GROUP, 128], bf16)
                nc.sync.dma_start(xng[:], x_wrap[:, t0 : t0 + TILES_PER_GROUP, :])
                ohTg = ohT_pool.tile([128, GROUP], f8)
                nc.sync.dma_start(ohTg[:], ohT[:, lo : lo + GROUP])
                ohg = oh_pool.tile([128, TILES_PER_GROUP, G_LOC], f8)
                nc.sync.dma_start(ohg[:], oh_wrap[:, t0 : t0 + TILES_PER_GROUP, :])
                ohgs[g] = ohg

                for st in range(4):
                    z = z_pool.tile([128, 4, 128], f32)
                    prev_mm2 = None
                    for s in range(4):
                        t_loc = st * 4 + s  # tile within group
                        xTh = xT_half[t_loc // 8]
                        thl = t_loc % 8
                        mm1 = nc.tensor.matmul(
                            z[:, s, :],
                            xTh[:, thl * 128 : (thl + 1) * 128],
                            w1_sb[:],
                            start=True,
                            stop=False,
                        )
                        if prev_mm2 is not None:
                            # PSUM has_written bits are cleared bank-wide by a
                            # start=True matmul; keep each subtile's accumulate
                            # pair ordered before the next pair's start.
                            add_dep_helper(mm1.ins, prev_mm2.ins, sync=False,
                                           reason="psum start/accum ordering")
                        mm2 = nc.tensor.matmul(
                            z[:, s, :],
                            ohTg[:, t_loc * 128 : (t_loc + 1) * 128],
                            uk_sb[:],
                            start=False,
                            stop=True,
                        )
                        prev_mm2 = mm2

                    gw = gw_pool.tile([128, 4, 128], bf16)
                    nc.scalar.activation(gw[:], z[:], AF.Sigmoid)
                    nc.vector.tensor_tensor(
                        vg[:, toff + st * 4 : toff + st * 4 + 4, 0:128],
                        gw[:],
                        xng[:, st * 4 : st * 4 + 4, :],
                        op=ALU.mult,
                    )
                    t_sg = toff + st * 4
                    scr = scr_pool.tile([128, 4, 128], bf16)
                    nc.vector.tensor_tensor(scr[:], gw[:], gw[:], op=ALU.mult)
                    sfold = scr_pool.tile([128, 4, 64], bf16, tag="sfold")
                    nc.vector.tensor_tensor(
                        sfold[:], scr[:, :, 0:64], scr[:, :, 64:128], op=ALU.add
                    )
                    nc.vector.tensor_reduce(
                        n2g[:, t_sg : t_sg + 4],
                        sfold[:],
                        axis=mybir.AxisListType.X,
                        op=ALU.add,
                    )

            # norm + count columns in two halves so scatter can start earlier.
            half = tiles_sg // 2 if tiles_sg > 1 else tiles_sg
            for h0, h1 in ((0, half), (half, tiles_sg)):
                if h0 == h1:
                    continue
                nrm_ap = vg[:, h0:h1, 128:129].rearrange("p t o -> p (t o)")
                one_ap = vg[:, h0:h1, 129:130].rearrange("p t o -> p (t o)")
                nc.scalar.activation(nrm_ap, n2g[:, h0:h1], AF.Sqrt)
                nc.gpsimd.memset(one_ap, 1.0)

            last_sg = g1 == n_groups
            for g in range(g0, g1):
                toff = (g - g0) * TILES_PER_GROUP
                for t_loc in range(TILES_PER_GROUP):
                    par = (toff + t_loc) % 2
                    acc = A0 if par == 0 else A1
                    nc.tensor.matmul(
                        acc[:, 0:130],
                        ohgs[g][:, t_loc, :],
                        vg[:, toff + t_loc, :],
                        start=first_mm3[par],
                        stop=last_sg
                        and g == g1 - 1
                        and t_loc >= TILES_PER_GROUP - 2,
                    )
                    first_mm3[par] = False

        # ---- merge the two accumulators ----
        A1c = sb_pool.tile([G_LOC, 130], f32, tag="a1c")
        nc.vector.tensor_copy(A1c[:], A1[:])
        As = sb_pool.tile([G_LOC, 130], f32, tag="asum")
        nc.vector.tensor_tensor(As[:], A0[:], A1c[:], op=ALU.add)

        # ---- GRU (PyTorch GRUCell math) on this core's 128 graphs ----
        gfn16 = sb_pool.tile([G_LOC, D], bf16, tag="gfn16")
        nc.vector.tensor_copy(gfn16[:], As[:, 0:128])
        tr_ps = g_pool.tile([D, G_LOC], bf16, tag="gru")
        nc.tensor.transpose(tr_ps[:], gfn16[:], ident_sb[:])
        gfnT = sb_pool.tile([D, G_LOC], bf16, tag="gfnT")
        nc.vector.tensor_copy(gfnT[:], tr_ps[:])

        gates = []
        for c in (0, 1):  # r, z
            ps = g_pool.tile([128, G_LOC], f32, tag="gru")
            nc.tensor.matmul(ps[:], wihT_sb[:, c * 128 : (c + 1) * 128], gfnT[:],
                             start=True, stop=False)
            nc.tensor.matmul(ps[:], whhT_sb[:, c * 128 : (c + 1) * 128], gkT16_sb[:],
                             start=False, stop=True)
            gate = sb_pool.tile([128, G_LOC], f32, tag=f"gate{c}")
            nc.scalar.activation(gate[:], ps[:], AF.Sigmoid, bias=bias_sb[:, c : c + 1])
            gates.append(gate)
        r_sb, z_sb = gates

        ps_in = g_pool.tile([128, G_LOC], f32, tag="gru")
        nc.tensor.matmul(ps_in[:], wihT_sb[:, 256:384], gfnT[:], start=True, stop=True)
        ps_hn = g_pool.tile([128, G_LOC], f32, tag="gru")
        nc.tensor.matmul(ps_hn[:], whhT_sb[:, 256:384], gkT16_sb[:], start=True, stop=True)

        hnb = sb_pool.tile([128, G_LOC], f32, tag="hnb")
        nc.vector.tensor_scalar(hnb[:], ps_hn[:], bias_sb[:, 2:3], None, op0=ALU.add)
        rh = sb_pool.tile([128, G_LOC], f32, tag="rh")
        nc.vector.tensor_tensor(rh[:], r_sb[:], hnb[:], op=ALU.mult)
        tsum = sb_pool.tile([128, G_LOC], f32, tag="tsum")
        nc.vector.tensor_tensor(tsum[:], rh[:], ps_in[:], op=ALU.add)
        n_sb = sb_pool.tile([128, G_LOC], f32, tag="nsb")
        nc.scalar.activation(n_sb[:], tsum[:], AF.Tanh, bias=bias_sb[:, 3:4])

        d1 = sb_pool.tile([128, G_LOC], f32, tag="d1")
        nc.vector.tensor_tensor(d1[:], gkTf_sb[:], n_sb[:], op=ALU.subtract)
        d2 = sb_pool.tile([128, G_LOC], f32, tag="d2")
        nc.vector.tensor_tensor(d2[:], z_sb[:], d1[:], op=ALU.mult)
        gfT_sb = sb_pool.tile([128, G_LOC], f32, tag="gfT")
        nc.vector.tensor_tensor(gfT_sb[:], n_sb[:], d2[:], op=ALU.add)
        nc.sync.dma_start(gfT_out, gfT_sb[:])

        # ---- gate-loss partial: sum_g seg_norm / max(counts, 1) ----
        cnt_m = sb_pool.tile([128, 1], f32, tag="cntm")
        nc.vector.tensor_scalar_max(cnt_m[:], As[:, 129:130], 1.0)
        rec = sb_pool.tile([128, 1], f32, tag="rec")
        nc.vector.reciprocal(rec[:], cnt_m[:])
        ratio = sb_pool.tile([128, 1], f32, tag="ratio")
        nc.vector.tensor_tensor(ratio[:], As[:, 128:129], rec[:], op=ALU.mult)
        loss_ps = g_pool.tile([1, 1], f32, tag="gru")
        nc.tensor.matmul(loss_ps[:], ratio[:], ones_sb[:], start=True, stop=True)
        loss_sb = sb_pool.tile([1, 1], f32, tag="losssb")
        nc.vector.tensor_copy(loss_sb[:], loss_ps[:])
        nc.sync.dma_start(loss_out, loss_sb[:])

    nc.compile()
    return nc


def _prepare_inputs(x, batch, global_information, mlp_w, mlp_b, w_ih, w_hh, b_ih, b_hh):
    """Shard by graph id, pad, cast; returns (nmax, list-of-8 input dicts)."""
    x = np.asarray(x)
    batch = np.asarray(batch)
    gi = np.asarray(global_information, dtype=np.float32)
    mlp_w = np.asarray(mlp_w, dtype=np.float32)
    mlp_b = np.asarray(mlp_b, dtype=np.float32)
    w_ih = np.asarray(w_ih, dtype=np.float32)
    w_hh = np.asarray(w_hh, dtype=np.float32)
    b_ih = np.asarray(b_ih, dtype=np.float32)
    b_hh = np.asarray(b_hh, dtype=np.float32)

    b64 = batch.astype(np.int64)
    # U = G_info @ W2 + b   (W2 = mlp_w.T[D:, :]); W1 = mlp_w.T[:D, :]
    w1_np = np.ascontiguousarray(mlp_w[:, :D].T)
    w2_np = mlp_w[:, D:].T
    u_full = gi.astype(np.float32) @ w2_np + mlp_b[None, :]

    core_of = b64 & 7
    g_loc_all = (b64 >> 3).astype(np.int16)
    counts = np.bincount(core_of, minlength=N_CORES)
    nmax = max(GROUP, _round_up(int(counts.max()), GROUP))

    bias4 = np.stack(
        [
            b_ih[0:128] + b_hh[0:128],
            b_ih[128:256] + b_hh[128:256],
            b_hh[256:384],
            b_ih[256:384],
        ],
        axis=1,
    ).astype(np.float32)

    ident = np.eye(128, dtype=BF16)
    ones_col = np.ones((128, 1), dtype=np.float32)
    wihT_np = np.ascontiguousarray(w_ih.T).astype(BF16)
    whhT_np = np.ascontiguousarray(w_hh.T).astype(BF16)

    in_maps = []
    for k in range(N_CORES):
        idx = np.nonzero(core_of == k)[0]
        cnt = idx.size
        x_k = np.zeros((nmax, D), dtype=BF16)
        x_k[:cnt] = x[idx]
        xT_np = np.ascontiguousarray(x_k.T)
        x_wrap = np.ascontiguousarray(
            x_k.reshape(nmax // 128, 128, D).transpose(1, 0, 2))
        gl = g_loc_all[idx].astype(np.int64)
        oh_nat = np.zeros((nmax, G_LOC), dtype=ml_dtypes.float8_e4m3)
        oh_nat[np.arange(cnt), gl] = 1.0
        oh_wrap = np.ascontiguousarray(
            oh_nat.reshape(nmax // 128, 128, G_LOC).transpose(1, 0, 2))
        ohT_np = np.zeros((128, nmax), dtype=ml_dtypes.float8_e4m3)
        ohT_np[gl, np.arange(cnt)] = 1.0
        gk = gi[k::8]  # [128, D] this core's graph states
        in_maps.append(
            {
                "xT": xT_np,
                "x_wrap": x_wrap,
                "oh_wrap": oh_wrap,
                "ohT": ohT_np,
                "u_k": u_full[k::8].astype(BF16),
                "w1": w1_np.astype(BF16),
                "gkT16": np.ascontiguousarray(gk.T).astype(BF16),
                "gkTf": np.ascontiguousarray(gk.T).astype(np.float32),
                "wihT": wihT_np,
                "whhT": whhT_np,
                "biases": bias4,
                "ident": ident,
                "ones_col": ones_col,
            }
        )
    return nmax, in_maps


def kernel(x, batch, global_information, mlp_w, mlp_b, w_ih, w_hh, b_ih, b_hh):
    from concourse import bass_utils

    nmax, in_maps = _prepare_inputs(
        x, batch, global_information, mlp_w, mlp_b, w_ih, w_hh, b_ih, b_hh
    )
    nc = _build_program(nmax)
    res = bass_utils.run_bass_kernel_spmd(
        nc,
        in_maps,
        core_ids=list(range(N_CORES)),
        trace=_PROFILE["trace"],
    )
    _PROFILE["last"] = res

    gf = np.empty((G, D), dtype=np.float32)
    loss_sum = 0.0
    for k in range(N_CORES):
        gf[k::8] = res.results[k]["gfT"].T
        loss_sum += float(res.results[k]["loss"][0, 0])
    gate_loss = np.float32(loss_sum / G)
    return gf, gate_loss
